# revision 6
# baseline (speedup 1.0000x reference)
"""Bahdanau additive attention kernel for Trainium2 (8 NeuronCores).

Problem: B=32, S=2048, H=1024
  k = key.transpose(1,0,2)                      # (B,S,H)
  e = tanh(query @ Wa + Wa_b + k @ Ua + Ua_b)   # (B,S,H)
  scores = e @ va_w (+ va_b, softmax-invariant) # (B,S)
  weights = softmax(scores, -1)[:, None, :]     # (B,1,S)
  context = weights @ k                         # (B,1,H)

Sharding: data-parallel over batch, 4 batches per core, params replicated.

Per-core dataflow (all matmul inputs bf16, accumulation fp32):
  - key slice loaded once via gpsimd cast-DMA into resident SBUF "friendly"
    tiles K[s=128, h] (16 MB bf16)
  - xbar DMA-transpose produces K^T[h=128, s] chunks for the PE contraction
  - E = K @ Ua on TensorE: stationary = K^T tile, moving = Ua tiles,
    bias c_b = q@Wa + Wa_b + Ua_b added via a K=1 rank-1 matmul
  - tanh on ScalarE (PSUM -> SBUF bf16)
  - scores = sum_h va[h] * tanh(...) via fused VectorE tensor_tensor_reduce
  - softmax without max-subtraction (tanh-bounded scores), Z via PE matvec,
    reciprocal on VectorE, partition_broadcast on GpSimd
  - context = sum_s w[s] K[s, h] on TensorE from the resident friendly tiles
  - weights written out via a PE transpose to make s contiguous in DRAM
"""

import numpy as np

import concourse.bacc as bacc
import concourse.mybir as mybir
import concourse.tile as tile
from concourse.bass_utils import run_bass_kernel_spmd
from concourse.masks import make_identity

F32 = mybir.dt.float32
BF16 = mybir.dt.bfloat16

N_CORES = 8
B, S, H = 32, 2048, 1024
B_LOCAL = B // N_CORES

_NC_CACHE = {}


def build_nc(b_local=B_LOCAL, s=S, h=H, n_cores=N_CORES):
    key = (b_local, s, h, n_cores)
    if key in _NC_CACHE:
        return _NC_CACHE[key]

    P = 128
    ST = s // P          # number of 128-row s-tiles per batch
    HT = h // P          # number of 128-row h-tiles
    NH = h // 512        # number of 512-wide output column halves
    SQ = min(4, ST)      # s-tiles per transpose chunk
    NSQ = ST // SQ

    nc = bacc.Bacc("TRN2", target_bir_lowering=False, debug=False,
                   num_devices=n_cores)

    key_d = nc.dram_tensor("key_sh", [s, b_local, h], F32, kind="ExternalInput")
    q_d = nc.dram_tensor("query_sh", [b_local, h], F32, kind="ExternalInput")
    wa_d = nc.dram_tensor("Wa_w", [h, h], F32, kind="ExternalInput")
    wab_d = nc.dram_tensor("Wa_b", [h], F32, kind="ExternalInput")
    ua_d = nc.dram_tensor("Ua_w", [h, h], F32, kind="ExternalInput")
    uab_d = nc.dram_tensor("Ua_b", [h], F32, kind="ExternalInput")
    va_d = nc.dram_tensor("va_w", [h], F32, kind="ExternalInput")
    ctx_d = nc.dram_tensor("ctx_out", [b_local, h], F32, kind="ExternalOutput")
    wout_d = nc.dram_tensor("w_out", [b_local, s], F32, kind="ExternalOutput")

    AF = mybir.ActivationFunctionType
    ALU = mybir.AluOpType

    with tile.TileContext(nc) as tc:
        with (
            tc.tile_pool(name="consts", bufs=1) as consts,
            tc.tile_pool(name="kres", bufs=1) as kres,
            tc.tile_pool(name="kt", bufs=2) as ktp,
            tc.tile_pool(name="work", bufs=1) as work,
            tc.tile_pool(name="tbuf", bufs=2) as tbuf,
            tc.tile_pool(name="pe_ps", bufs=2, space="PSUM") as pe_ps,
            tc.tile_pool(name="cps", bufs=1, space="PSUM") as cps,
            tc.tile_pool(name="sps", bufs=1, space="PSUM") as sps,
        ):
            # ---------- phase 0: constants & q-projection ----------
            ident4 = consts.tile([b_local, b_local], F32, tag="ident4")
            make_identity(nc, ident4[:])
            ident128 = consts.tile([P, P], F32, tag="ident128")
            make_identity(nc, ident128[:])
            ones_row = consts.tile([1, P], BF16, tag="ones_row")
            nc.vector.memset(ones_row[:], 1.0)
            ones4 = consts.tile([1, b_local], BF16, tag="ones4")
            nc.vector.memset(ones4[:], 1.0)
            ones_col = consts.tile([P, 1], F32, tag="ones_col")
            nc.vector.memset(ones_col[:], 1.0)

            # bias row: Wa_b + Ua_b, bf16
            wab = work.tile([1, h], F32, tag="s4k_a")
            nc.sync.dma_start(out=wab[:], in_=wab_d.ap())
            uab = work.tile([1, h], F32, tag="s4k_b")
            nc.sync.dma_start(out=uab[:], in_=uab_d.ap())
            bias_bf = consts.tile([1, h], BF16, tag="bias_bf")
            nc.vector.tensor_tensor(out=bias_bf[:], in0=wab[:], in1=uab[:],
                                    op=ALU.add)

            # va replicated across partitions, bf16
            va_row = work.tile([1, h], BF16, tag="s2k")
            nc.gpsimd.dma_start(out=va_row[:], in_=va_d.ap())  # casts f32->bf16
            va_rep = consts.tile([P, h], BF16, tag="va_rep")
            nc.gpsimd.partition_broadcast(va_rep[:], va_row[:])

            # q^T tiles: [h-part, HT, b] bf16 via PE transpose
            qf = work.tile([b_local, h], F32, tag="s4k_a")
            nc.sync.dma_start(out=qf[:], in_=q_d.ap())
            qt_ps = sps.tile([P, HT, b_local], F32, tag="sps")
            for t in range(HT):
                nc.tensor.transpose(qt_ps[:, t, :], qf[:, t * P:(t + 1) * P],
                                    ident4[:])
            qt_bf = consts.tile([P, HT, b_local], BF16, tag="qt_bf")
            nc.vector.tensor_copy(qt_bf[:], qt_ps[:])

            # c = q @ Wa + bias; Wa streamed in bf16 chunks of 128 rows
            c_ps = cps.tile([b_local, h], F32, tag="cps")
            for half in range(NH):
                sl = slice(512 * half, 512 * half + 512)
                nc.tensor.matmul(c_ps[:, sl], lhsT=ones4[:], rhs=bias_bf[:, sl],
                                 start=True, stop=False)
            for t in range(HT):
                wa_ch = tbuf.tile([P, h], BF16, tag="t_sb", name="wa_ch")
                nc.gpsimd.dma_start(out=wa_ch[:],
                                    in_=wa_d.ap()[t * P:(t + 1) * P, :])
                for half in range(NH):
                    sl = slice(512 * half, 512 * half + 512)
                    nc.tensor.matmul(c_ps[:, sl], lhsT=qt_bf[:, t, :],
                                     rhs=wa_ch[:, sl],
                                     start=False, stop=(t == HT - 1))
            c_sb4 = work.tile([b_local, h], BF16, tag="s2k")
            nc.scalar.copy(c_sb4[:], c_ps[:])
            # move each batch's bias row to partition 0 (SBUF->SBUF DMA)
            c_rows = consts.tile([1, b_local, h], BF16, tag="c_rows")
            for b in range(b_local):
                nc.sync.dma_start(out=c_rows[:, b, :], in_=c_sb4[b:b + 1, :])

            # Ua (bf16): [h_in-part, HT, h_out]
            ua_bf = consts.tile([P, HT, h], BF16, tag="ua_bf")
            nc.gpsimd.dma_start(
                out=ua_bf[:],
                in_=ua_d.ap().rearrange("(t p) f -> p t f", p=P))

            # scores accumulator: [s-part, b, ST] fp32
            scores = consts.tile([P, b_local, ST], F32, tag="scores")

            # resident friendly key tiles, one per local batch
            kres_b = [kres.tile([P, ST, h], BF16, tag=f"kres{b}", name=f"kres{b}")
                      for b in range(b_local)]

            # ---------- phase 1: main pipeline ----------
            for b in range(b_local):
                kin = key_d.ap()[:, b, :].rearrange("(so p) f -> p so f", p=P)
                for sq in range(NSQ):
                    ssl = slice(sq * SQ, (sq + 1) * SQ)
                    # friendly load (cast f32 -> bf16 in the DMA)
                    nc.gpsimd.dma_start(out=kres_b[b][:, ssl, :],
                                        in_=kin[:, ssl, :])
                    # K^T chunk via xbar transpose: rows (so_l, ht, hp)
                    kt = ktp.tile([P, SQ, HT, P], BF16, tag="kt")
                    nc.sync.dma_start(out=kt[:], in_=kres_b[b][:, ssl, :],
                                      transpose=True)
                    for so_l in range(SQ):
                        so = sq * SQ + so_l
                        e_ps = pe_ps.tile([P, h], F32, tag="e_ps")
                        for half in range(NH):
                            sl = slice(512 * half, 512 * half + 512)
                            nc.tensor.matmul(e_ps[:, sl], lhsT=ones_row[:],
                                             rhs=c_rows[:, b, sl],
                                             start=True, stop=False)
                            for t in range(HT):
                                nc.tensor.matmul(e_ps[:, sl],
                                                 lhsT=kt[:, so_l, t, :],
                                                 rhs=ua_bf[:, t, sl],
                                                 start=False,
                                                 stop=(t == HT - 1))
                        t_sb = tbuf.tile([P, h], BF16, tag="t_sb")
                        nc.scalar.activation(t_sb[:], e_ps[:], AF.Tanh)
                        prod = work.tile([P, h], BF16, tag="prod")
                        nc.vector.scalar_tensor_tensor(
                            out=prod[:], in0=t_sb[:], scalar=1.0,
                            in1=va_rep[:],
                            op0=ALU.mult, op1=ALU.mult,
                            accum_out=scores[:, b, so:so + 1])

            # ---------- phase 2: softmax + weights output ----------
            w_bf_all = consts.tile([P, b_local, ST], BF16, tag="w_bf_all")
            for b in range(b_local):
                p_f = work.tile([P, ST], F32, tag="p_f")
                rowsum = work.tile([P, 1], F32, tag="rowsum")
                nc.scalar.activation(p_f[:], scores[:, b, :], AF.Exp,
                                     accum_out=rowsum[:])
                z_ps = sps.tile([1, 1], F32, tag="sps")
                nc.tensor.matmul(z_ps[:], lhsT=rowsum[:], rhs=ones_col[:],
                                 start=True, stop=True)
                invz = work.tile([1, 1], F32, tag="invz")
                nc.vector.reciprocal(invz[:], z_ps[:])
                invz_rep = work.tile([P, 1], F32, tag="invz_rep")
                nc.gpsimd.partition_broadcast(invz_rep[:], invz[:])
                w_f = work.tile([P, ST], F32, tag="w_f")
                nc.vector.tensor_scalar_mul(w_f[:], p_f[:], invz_rep[:])
                nc.vector.tensor_copy(w_bf_all[:, b, :], w_f[:])
                # weights out: transpose [s-part, ST] -> [ST, s-inner]
                wt_ps = sps.tile([ST, P], F32, tag="sps")
                nc.tensor.transpose(wt_ps[:], w_f[:], ident128[:])
                wt_sb = work.tile([ST, P], F32, tag="s4k_b", name="wt_sb")
                nc.scalar.copy(wt_sb[:], wt_ps[:])
                nc.sync.dma_start(
                    out=wout_d.ap()[b].rearrange("(a p) -> a p", p=P),
                    in_=wt_sb[:])

            # ---------- phase 3: context ----------
            for b in range(b_local):
                ctx_ps = cps.tile([1, h], F32, tag="cps")
                for so in range(ST):
                    for half in range(NH):
                        sl = slice(512 * half, 512 * half + 512)
                        nc.tensor.matmul(ctx_ps[:, sl],
                                         lhsT=w_bf_all[:, b, so:so + 1],
                                         rhs=kres_b[b][:, so, sl],
                                         start=(so == 0), stop=(so == ST - 1))
                ctx_sb = work.tile([1, h], F32, tag="s4k_a", name="ctx_sb")
                nc.scalar.copy(ctx_sb[:], ctx_ps[:])
                nc.sync.dma_start(out=ctx_d.ap()[b], in_=ctx_sb[:])

    nc.compile()
    _NC_CACHE[key] = nc
    return nc


def build_in_maps(query, key, Wa_w, Wa_b, Ua_w, Ua_b, va_w, va_b=None):
    query = np.asarray(query, dtype=np.float32)
    key = np.asarray(key, dtype=np.float32)
    shared = {
        "Wa_w": np.ascontiguousarray(np.asarray(Wa_w, np.float32)),
        "Wa_b": np.ascontiguousarray(np.asarray(Wa_b, np.float32)),
        "Ua_w": np.ascontiguousarray(np.asarray(Ua_w, np.float32)),
        "Ua_b": np.ascontiguousarray(np.asarray(Ua_b, np.float32)),
        "va_w": np.ascontiguousarray(np.asarray(va_w, np.float32)),
    }
    in_maps = []
    for i in range(N_CORES):
        bs = slice(i * B_LOCAL, (i + 1) * B_LOCAL)
        in_maps.append(dict(
            shared,
            key_sh=np.ascontiguousarray(key[:, bs, :]),
            query_sh=np.ascontiguousarray(query[bs, 0, :]),
        ))
    return in_maps


def gather_outputs(results):
    context = np.concatenate([r["ctx_out"] for r in results], axis=0)
    weights = np.concatenate([r["w_out"] for r in results], axis=0)
    return context[:, None, :], weights[:, None, :]


def kernel(query, key, Wa_w, Wa_b, Ua_w, Ua_b, va_w, va_b):
    nc = build_nc()
    in_maps = build_in_maps(query, key, Wa_w, Wa_b, Ua_w, Ua_b, va_w, va_b)
    res = run_bass_kernel_spmd(nc, in_maps, core_ids=list(range(N_CORES)))
    return gather_outputs(res.results)


# revision 8
# speedup vs baseline: 1.1810x; 1.1810x over previous
"""Bahdanau additive attention kernel for Trainium2 (8 NeuronCores).

Problem: B=32, S=2048, H=1024
  k = key.transpose(1,0,2)                      # (B,S,H)
  e = tanh(query @ Wa + Wa_b + k @ Ua + Ua_b)   # (B,S,H)
  scores = e @ va_w (+ va_b, softmax-invariant) # (B,S)
  weights = softmax(scores, -1)[:, None, :]     # (B,1,S)
  context = weights @ k                         # (B,1,H)

Sharding: data-parallel over batch, 4 batches per core, params replicated.

Per-core dataflow (all matmul inputs bf16, accumulation fp32):
  - key slice loaded once via gpsimd cast-DMA into resident SBUF "friendly"
    tiles K[s=128, h] (16 MB bf16)
  - xbar DMA-transpose produces K^T[h=128, s] chunks for the PE contraction
  - E = K @ Ua on TensorE: stationary = K^T tile, moving = Ua tiles,
    bias c_b = q@Wa + Wa_b + Ua_b added via a K=1 rank-1 matmul
  - tanh on ScalarE (PSUM -> SBUF bf16)
  - scores = sum_h va[h] * tanh(...) via fused VectorE tensor_tensor_reduce
  - softmax without max-subtraction (tanh-bounded scores), Z via PE matvec,
    reciprocal on VectorE, partition_broadcast on GpSimd
  - context = sum_s w[s] K[s, h] on TensorE from the resident friendly tiles
  - weights written out via a PE transpose to make s contiguous in DRAM
"""

import numpy as np

import concourse.bacc as bacc
import concourse.mybir as mybir
import concourse.tile as tile
from concourse.bass_utils import run_bass_kernel_spmd
from concourse.masks import make_identity

F32 = mybir.dt.float32
BF16 = mybir.dt.bfloat16

N_CORES = 8
B, S, H = 32, 2048, 1024
B_LOCAL = B // N_CORES

_NC_CACHE = {}


def build_nc(b_local=B_LOCAL, s=S, h=H, n_cores=N_CORES):
    key = (b_local, s, h, n_cores)
    if key in _NC_CACHE:
        return _NC_CACHE[key]

    P = 128
    ST = s // P          # number of 128-row s-tiles per batch
    HT = h // P          # number of 128-row h-tiles
    NH = h // 512        # number of 512-wide output column halves
    SQ = min(4, ST)      # s-tiles per transpose chunk
    NSQ = ST // SQ

    nc = bacc.Bacc("TRN2", target_bir_lowering=False, debug=False,
                   num_devices=n_cores)

    key_d = nc.dram_tensor("key_sh", [s, b_local, h], F32, kind="ExternalInput")
    q_d = nc.dram_tensor("query_sh", [b_local, h], F32, kind="ExternalInput")
    wa_d = nc.dram_tensor("Wa_w", [h, h], F32, kind="ExternalInput")
    wab_d = nc.dram_tensor("Wa_b", [h], F32, kind="ExternalInput")
    ua_d = nc.dram_tensor("Ua_w", [h, h], F32, kind="ExternalInput")
    uab_d = nc.dram_tensor("Ua_b", [h], F32, kind="ExternalInput")
    va_d = nc.dram_tensor("va_w", [h], F32, kind="ExternalInput")
    ctx_d = nc.dram_tensor("ctx_out", [b_local, h], F32, kind="ExternalOutput")
    wout_d = nc.dram_tensor("w_out", [b_local, s], F32, kind="ExternalOutput")

    AF = mybir.ActivationFunctionType
    ALU = mybir.AluOpType

    with tile.TileContext(nc) as tc:
        with (
            tc.tile_pool(name="consts", bufs=1) as consts,
            tc.tile_pool(name="kres", bufs=1) as kres,
            tc.tile_pool(name="kt", bufs=2) as ktp,
            tc.tile_pool(name="work", bufs=1) as work,
            tc.tile_pool(name="tbuf", bufs=2) as tbuf,
            tc.tile_pool(name="pe_ps", bufs=2, space="PSUM") as pe_ps,
            tc.tile_pool(name="cps", bufs=1, space="PSUM") as cps,
            tc.tile_pool(name="sps", bufs=1, space="PSUM") as sps,
        ):
            # ---------- phase 0: constants & q-projection ----------
            ident4 = consts.tile([b_local, b_local], F32, tag="ident4")
            make_identity(nc, ident4[:])
            ident128 = consts.tile([P, P], F32, tag="ident128")
            make_identity(nc, ident128[:])
            ones_row = consts.tile([1, P], BF16, tag="ones_row")
            nc.vector.memset(ones_row[:], 1.0)
            ones4 = consts.tile([1, b_local], BF16, tag="ones4")
            nc.vector.memset(ones4[:], 1.0)
            ones_col = consts.tile([P, 1], F32, tag="ones_col")
            nc.vector.memset(ones_col[:], 1.0)

            # Ua (bf16): [h_in-part, HT, h_out] -- first on the SWDGE queue,
            # it gates the first main matmul
            ua_bf = consts.tile([P, HT, h], BF16, tag="ua_bf")
            nc.gpsimd.dma_start(
                out=ua_bf[:],
                in_=ua_d.ap().rearrange("(t p) f -> p t f", p=P))

            # resident friendly key tiles, one per local batch; batch 0 loads
            # queue before the q-projection weights
            kres_b = [kres.tile([P, ST, h], BF16, tag=f"kres{b}", name=f"kres{b}")
                      for b in range(b_local)]
            kins = [key_d.ap()[:, b, :].rearrange("(so p) f -> p so f", p=P)
                    for b in range(b_local)]
            for sq in range(NSQ):
                ssl = slice(sq * SQ, (sq + 1) * SQ)
                nc.gpsimd.dma_start(out=kres_b[0][:, ssl, :],
                                    in_=kins[0][:, ssl, :])

            # bias row: Wa_b + Ua_b, bf16
            wab = work.tile([1, h], F32, tag="s4k_a")
            nc.sync.dma_start(out=wab[:], in_=wab_d.ap())
            uab = work.tile([1, h], F32, tag="s4k_b")
            nc.sync.dma_start(out=uab[:], in_=uab_d.ap())
            bias_bf = consts.tile([1, h], BF16, tag="bias_bf")
            nc.vector.tensor_tensor(out=bias_bf[:], in0=wab[:], in1=uab[:],
                                    op=ALU.add)

            # va replicated across partitions, bf16
            va_row = work.tile([1, h], BF16, tag="s2k")
            nc.gpsimd.dma_start(out=va_row[:], in_=va_d.ap())  # casts f32->bf16
            va_rep = consts.tile([P, h], BF16, tag="va_rep")
            nc.gpsimd.partition_broadcast(va_rep[:], va_row[:])

            # q^T tiles: [h-part, HT, b] bf16 via PE transpose
            qf = work.tile([b_local, h], F32, tag="s4k_a")
            nc.sync.dma_start(out=qf[:], in_=q_d.ap())
            qt_ps = sps.tile([P, HT, b_local], F32, tag="sps")
            for t in range(HT):
                nc.tensor.transpose(qt_ps[:, t, :], qf[:, t * P:(t + 1) * P],
                                    ident4[:])
            qt_bf = consts.tile([P, HT, b_local], BF16, tag="qt_bf")
            nc.vector.tensor_copy(qt_bf[:], qt_ps[:])

            # c = q @ Wa + bias; Wa streamed in bf16 chunks of 128 rows
            c_ps = cps.tile([b_local, h], F32, tag="cps")
            for half in range(NH):
                sl = slice(512 * half, 512 * half + 512)
                nc.tensor.matmul(c_ps[:, sl], lhsT=ones4[:], rhs=bias_bf[:, sl],
                                 start=True, stop=False)
            for t in range(HT):
                wa_ch = tbuf.tile([P, h], BF16, tag="t_sb", name="wa_ch")
                nc.gpsimd.dma_start(out=wa_ch[:],
                                    in_=wa_d.ap()[t * P:(t + 1) * P, :])
                for half in range(NH):
                    sl = slice(512 * half, 512 * half + 512)
                    nc.tensor.matmul(c_ps[:, sl], lhsT=qt_bf[:, t, :],
                                     rhs=wa_ch[:, sl],
                                     start=False, stop=(t == HT - 1))
            c_sb4 = work.tile([b_local, h], BF16, tag="s2k")
            nc.scalar.copy(c_sb4[:], c_ps[:])
            # move each batch's bias row to partition 0 (SBUF->SBUF DMA)
            c_rows = consts.tile([1, b_local, h], BF16, tag="c_rows")
            for b in range(b_local):
                nc.sync.dma_start(out=c_rows[:, b, :], in_=c_sb4[b:b + 1, :])

            # scores accumulator: [s-part, b, ST] fp32
            scores = consts.tile([P, b_local, ST], F32, tag="scores")

            # ---------- main pipeline, per batch ----------
            w_bf_all = consts.tile([P, b_local, ST], BF16, tag="w_bf_all")
            for b in range(b_local):
                for sq in range(NSQ):
                    ssl = slice(sq * SQ, (sq + 1) * SQ)
                    # friendly load (cast f32 -> bf16 in the DMA);
                    # batch 0 chunks were queued during setup
                    if b > 0:
                        nc.gpsimd.dma_start(out=kres_b[b][:, ssl, :],
                                            in_=kins[b][:, ssl, :])
                    # K^T chunk via xbar transpose: rows (so_l, ht, hp)
                    kt = ktp.tile([P, SQ, HT, P], BF16, tag="kt")
                    nc.sync.dma_start(out=kt[:], in_=kres_b[b][:, ssl, :],
                                      transpose=True)
                    for so_l in range(SQ):
                        so = sq * SQ + so_l
                        e_ps = pe_ps.tile([P, h], F32, tag="e_ps")
                        for half in range(NH):
                            sl = slice(512 * half, 512 * half + 512)
                            for t in range(HT):
                                nc.tensor.matmul(e_ps[:, sl],
                                                 lhsT=kt[:, so_l, t, :],
                                                 rhs=ua_bf[:, t, sl],
                                                 start=(t == 0),
                                                 stop=False)
                            # bias last: rank-1 add of c_b (keeps the first
                            # matmuls off the q-projection critical path)
                            nc.tensor.matmul(e_ps[:, sl], lhsT=ones_row[:],
                                             rhs=c_rows[:, b, sl],
                                             start=False, stop=True)
                        t_sb = tbuf.tile([P, h], BF16, tag="t_sb")
                        nc.scalar.activation(t_sb[:], e_ps[:], AF.Tanh)
                        prod = work.tile([P, h], BF16, tag="prod")
                        nc.vector.scalar_tensor_tensor(
                            out=prod[:], in0=t_sb[:], scalar=1.0,
                            in1=va_rep[:],
                            op0=ALU.mult, op1=ALU.mult,
                            accum_out=scores[:, b, so:so + 1])

                # ---- softmax + weights output + context for this batch ----
                p_f = work.tile([P, ST], F32, tag="p_f")
                rowsum = work.tile([P, 1], F32, tag="rowsum")
                nc.scalar.activation(p_f[:], scores[:, b, :], AF.Exp,
                                     accum_out=rowsum[:])
                z_ps = sps.tile([1, 1], F32, tag="sps")
                nc.tensor.matmul(z_ps[:], lhsT=rowsum[:], rhs=ones_col[:],
                                 start=True, stop=True)
                invz = work.tile([1, 1], F32, tag="invz")
                nc.vector.reciprocal(invz[:], z_ps[:])
                invz_rep = work.tile([P, 1], F32, tag="invz_rep")
                nc.gpsimd.partition_broadcast(invz_rep[:], invz[:])
                w_f = work.tile([P, ST], F32, tag="w_f")
                nc.vector.tensor_scalar_mul(w_f[:], p_f[:], invz_rep[:])
                nc.vector.tensor_copy(w_bf_all[:, b, :], w_f[:])
                # weights out: transpose [s-part, ST] -> [ST, s-inner]
                wt_ps = sps.tile([ST, P], F32, tag="sps")
                nc.tensor.transpose(wt_ps[:], w_f[:], ident128[:])
                wt_sb = work.tile([ST, P], F32, tag="s4k_b", name="wt_sb")
                nc.scalar.copy(wt_sb[:], wt_ps[:])
                nc.sync.dma_start(
                    out=wout_d.ap()[b].rearrange("(a p) -> a p", p=P),
                    in_=wt_sb[:])

                ctx_ps = cps.tile([1, h], F32, tag="cps")
                for so in range(ST):
                    for half in range(NH):
                        sl = slice(512 * half, 512 * half + 512)
                        nc.tensor.matmul(ctx_ps[:, sl],
                                         lhsT=w_bf_all[:, b, so:so + 1],
                                         rhs=kres_b[b][:, so, sl],
                                         start=(so == 0), stop=(so == ST - 1))
                ctx_sb = work.tile([1, h], F32, tag="s4k_a", name="ctx_sb")
                nc.scalar.copy(ctx_sb[:], ctx_ps[:])
                nc.sync.dma_start(out=ctx_d.ap()[b], in_=ctx_sb[:])

    nc.compile()
    _NC_CACHE[key] = nc
    return nc


def build_in_maps(query, key, Wa_w, Wa_b, Ua_w, Ua_b, va_w, va_b=None):
    query = np.asarray(query, dtype=np.float32)
    key = np.asarray(key, dtype=np.float32)
    shared = {
        "Wa_w": np.ascontiguousarray(np.asarray(Wa_w, np.float32)),
        "Wa_b": np.ascontiguousarray(np.asarray(Wa_b, np.float32)),
        "Ua_w": np.ascontiguousarray(np.asarray(Ua_w, np.float32)),
        "Ua_b": np.ascontiguousarray(np.asarray(Ua_b, np.float32)),
        "va_w": np.ascontiguousarray(np.asarray(va_w, np.float32)),
    }
    in_maps = []
    for i in range(N_CORES):
        bs = slice(i * B_LOCAL, (i + 1) * B_LOCAL)
        in_maps.append(dict(
            shared,
            key_sh=np.ascontiguousarray(key[:, bs, :]),
            query_sh=np.ascontiguousarray(query[bs, 0, :]),
        ))
    return in_maps


def gather_outputs(results):
    context = np.concatenate([r["ctx_out"] for r in results], axis=0)
    weights = np.concatenate([r["w_out"] for r in results], axis=0)
    return context[:, None, :], weights[:, None, :]


def kernel(query, key, Wa_w, Wa_b, Ua_w, Ua_b, va_w, va_b):
    nc = build_nc()
    in_maps = build_in_maps(query, key, Wa_w, Wa_b, Ua_w, Ua_b, va_w, va_b)
    res = run_bass_kernel_spmd(nc, in_maps, core_ids=list(range(N_CORES)))
    return gather_outputs(res.results)
